# revision 1
# baseline (speedup 1.0000x reference)
"""Tensor-parallel Llama attention (decode, GQA, RoPE, KV-cache) on 8 TRN2 cores.

Sharding: core c owns kv-head c and q-heads 4c..4c+3. Wq/Wk/Wv are sharded
column-wise, Wo row-wise; each core computes a partial o_proj output and the
host sums the 8 partials (the all-reduce).

Per-core kernel layout notes:
  - Everything is kept "transposed" ([d, token] / [d, kpos]) so that every
    matmul contracts over the partition dim with M=128 (full PE array):
      qT/kT/vnew from projections, scoresT = kT_tile.T @ qT, attnT = v.T @ exp.
  - Softmax runs without max-subtraction (|score| <= ~8 here, exp is safe in
    fp32) so the kpos-partition layout only needs a sum: DVE accumulates exp
    tiles, a ones-column matmul reduces over partitions, and a 1x128 ones
    matmul broadcasts 1/denom back over partitions.
  - The causal mask only affects the 16 fresh keys (bottom-right aligned),
    applied as a 0/1 multiply on the one small fresh-score tile.
"""

import numpy as np
import ml_dtypes

import concourse.bass as bass
import concourse.mybir as mybir
import concourse.tile as tile
from concourse import bacc
from concourse.bass_utils import run_bass_kernel_spmd

F32 = mybir.dt.float32
BF16 = mybir.dt.bfloat16
AF = mybir.ActivationFunctionType

# Problem shape (hardcoded per contract)
B, S, H = 4, 16, 4096
NH, NKV, HD = 32, 8, 128
PAST = 8192
ROPE_BASE = 10000.0
NCORES = 8
HQ = NH // NCORES          # q heads per core = 4
TOK = B * S                # 64 tokens
NCH = H // 128             # 32 contraction chunks for projections
ROWS = HQ * S              # 64 (head, token) query rows per batch
SCALE = HD ** -0.5


def build_nc(b=B, s=S, h=H, hq=HQ, hd=HD, past=PAST):
    tok = b * s
    nch = h // 128
    rows = hq * s
    ktiles = past // 128
    halves = 2                      # stream k/v caches in 2 chunks per batch
    kt_half = ktiles // halves

    nc = bacc.Bacc("TRN2", target_bir_lowering=False, debug=False)

    hiddenT_d = nc.dram_tensor("hiddenT", [h, tok], BF16, kind="ExternalInput").ap()
    wq_d = nc.dram_tensor("wq", [h, hq * hd], BF16, kind="ExternalInput").ap()
    wkv_d = nc.dram_tensor("wkv", [h, 2 * hd], BF16, kind="ExternalInput").ap()
    wo_d = nc.dram_tensor("wo", [hq * hd, h], BF16, kind="ExternalInput").ap()
    kT_d = nc.dram_tensor("kT", [b, hd, past], BF16, kind="ExternalInput").ap()
    v_d = nc.dram_tensor("v", [b, 128, past], BF16, kind="ExternalInput").ap()
    cosT_d = nc.dram_tensor("cosT", [hd, tok], F32, kind="ExternalInput").ap()
    sinT_d = nc.dram_tensor("sinT", [hd, tok], F32, kind="ExternalInput").ap()
    nsinT_d = nc.dram_tensor("nsinT", [hd, tok], F32, kind="ExternalInput").ap()
    maskT_d = nc.dram_tensor("maskT", [s, rows], F32, kind="ExternalInput").ap()
    out_d = nc.dram_tensor("out_p", [tok, h], F32, kind="ExternalOutput").ap()

    with tile.TileContext(nc) as tc:
        import contextlib

        with contextlib.ExitStack() as ctx:
            ep = ctx.enter_context          # shorthand
            const_p = ep(tc.tile_pool(name="const", bufs=1))
            hT_p = ep(tc.tile_pool(name="hT", bufs=1))
            wq_p = ep(tc.tile_pool(name="wq", bufs=3))
            wkv_p = ep(tc.tile_pool(name="wkv", bufs=3))
            wo_p = ep(tc.tile_pool(name="wo", bufs=32))
            kv_p = ep(tc.tile_pool(name="kv", bufs=6))
            qkv_p = ep(tc.tile_pool(name="qkv", bufs=1))
            rope_p = ep(tc.tile_pool(name="rope", bufs=4))
            exp_p = ep(tc.tile_pool(name="exp", bufs=6))
            acc_p = ep(tc.tile_pool(name="acc", bufs=2))
            den_p = ep(tc.tile_pool(name="den", bufs=2))
            # PSUM: 8 banks total; tags share banks across phases:
            #   "A"(2): qt (proj) -> ops (o_proj);  "attn"(2): per-batch attn acc
            #   "B"(2): ktn+vn (proj) -> dsum/bc (softmax);  "sc"(2): score tiles
            ps = ep(tc.tile_pool(name="ps", bufs=2, space="PSUM"))

            # ---- constants ----
            ones_col = const_p.tile([128, 1], F32)
            nc.vector.memset(ones_col[:], 1.0)
            ones_row = const_p.tile([1, 128], F32)
            nc.vector.memset(ones_row[:], 1.0)
            cosT = const_p.tile([hd, tok], F32)
            nc.sync.dma_start(cosT[:], cosT_d[:])
            sinT = const_p.tile([hd, tok], F32)
            nc.sync.dma_start(sinT[:], sinT_d[:])
            nsinT = const_p.tile([hd, tok], F32)
            nc.sync.dma_start(nsinT[:], nsinT_d[:])
            maskT = const_p.tile([s, rows], F32)
            nc.sync.dma_start(maskT[:], maskT_d[:])
            ident = const_p.tile([tok, tok], F32)
            from concourse.masks import make_identity
            make_identity(nc, ident[:])

            # ---- load hiddenT: [h, tok] -> sbuf [128, nch*tok] ----
            hT = hT_p.tile([128, nch * tok], BF16)
            nc.sync.dma_start(
                hT[:].rearrange("p (c t) -> p c t", c=nch),
                hiddenT_d.rearrange("(c p) t -> p c t", p=128),
            )

            # ---- projections: qT_ps[j] [128, tok], kT_ps [128, tok], v_ps [tok, 128] ----
            # q in token-major [tok, hq*hd] (single PSUM bank/group); k/v direct
            q_ps = ps.tile([tok, hq * hd], F32, tag="A")
            kT_ps = ps.tile([128, tok], F32, tag="B")
            v_ps = ps.tile([tok, 128], F32, tag="B")
            for c in range(nch):
                wq_t = wq_p.tile([128, hq * hd], BF16)
                nc.sync.dma_start(
                    wq_t[:], wq_d.rearrange("(c p) m -> c p m", p=128)[c]
                )
                wkv_t = wkv_p.tile([128, 2 * hd], BF16)
                nc.sync.dma_start(
                    wkv_t[:], wkv_d.rearrange("(c p) m -> c p m", p=128)[c]
                )
                rhs_h = hT[:, c * tok:(c + 1) * tok]
                fl = dict(start=(c == 0), stop=(c == nch - 1))
                nc.tensor.matmul(q_ps[:], rhs_h, wq_t[:], **fl)
                nc.tensor.matmul(kT_ps[:], wkv_t[:, 0:hd], rhs_h, **fl)
                nc.tensor.matmul(v_ps[:], rhs_h, wkv_t[:, hd:2 * hd], **fl)
            q_sb = qkv_p.tile([tok, hq * hd], F32, tag="qsb")
            nc.scalar.copy(q_sb[:], q_ps[:])

            # ---- RoPE -> qT_sb [128, (b,hq,s)], kT_new [128, (b,s)], v_new [tok, 128] ----
            half = hd // 2
            qT_sb = qkv_p.tile([128, b * rows], F32, tag="qT")
            kT_new = qkv_p.tile([128, tok], F32, tag="kTn")
            # per-batch fresh-v tiles at base partition 0 (PE wants base 0/32/64)
            v_new = [
                qkv_p.tile([s, hd], F32, tag=f"vnew{bb}", name=f"vnew{bb}")
                for bb in range(b)
            ]

            def rope(dst, src_ps):
                # dst = src*cos + rotate_half(src)*sin  (all [128, tok], (b,t) cols)
                t1 = rope_p.tile([128, tok], F32, tag="r1")
                nc.vector.tensor_mul(t1[:], src_ps[:], cosT[:])
                t2 = rope_p.tile([128, tok], F32, tag="r2")
                nc.vector.tensor_mul(
                    t2[0:half, :], src_ps[half:hd, :], nsinT[0:half, :]
                )
                nc.vector.tensor_mul(
                    t2[half:hd, :], src_ps[0:half, :], sinT[half:hd, :]
                )
                nc.vector.tensor_add(dst, t1[:], t2[:])
                return dst

            for j in range(hq):
                # transpose head j to [d, (b,t)], then rope-scatter to (b, j, t)
                qt_ps = ps.tile([hd, tok], F32, tag="sc", name=f"qtp{j}")
                nc.tensor.transpose(
                    qt_ps[:], q_sb[:, j * hd:(j + 1) * hd], ident[:]
                )
                dst = qT_sb[:].rearrange("p (bb j t) -> p bb j t", bb=b, j=hq)[:, :, j, :]
                rope(dst, qt_ps)
            rope(kT_new[:], kT_ps)
            v_sb = qkv_p.tile([tok, hd], F32, tag="vsb")
            nc.scalar.copy(v_sb[:], v_ps[:])
            for bb in range(b):
                nc.sync.dma_start(v_new[bb][:], v_sb[bb * s:(bb + 1) * s, :])

            qT_bf = qkv_p.tile([128, b * rows], BF16, tag="qTbf")
            nc.vector.tensor_copy(qT_bf[:], qT_sb[:])

            # ---- attention per batch ----
            # Scores are built 8 kpos-tiles at a time into ONE psum bank
            # (disjoint column ranges, one accumulation group) so exp / the
            # denominator reduce run 512 wide, 8x fewer cross-engine hops.
            GRP = 512 // rows               # kpos tiles per score group (8)
            attnT_sb = qkv_p.tile([128, hq * tok], BF16, tag="attnT")  # (h, b, t) cols
            for bb in range(b):
                qT_b = qT_bf[:, bb * rows:(bb + 1) * rows]  # [128, (h,t)] bf16
                qT_b32 = qT_sb[:, bb * rows:(bb + 1) * rows]
                attn_ps = ps.tile([128, rows], F32, tag="attn")
                acc = acc_p.tile([128, rows], F32, tag="acc")
                for hf in range(halves):
                    kt = kv_p.tile([128, kt_half * 128], BF16, tag="kt")
                    nc.sync.dma_start(
                        kt[:], kT_d[bb, :, hf * kt_half * 128:(hf + 1) * kt_half * 128]
                    )
                    vt = kv_p.tile([128, kt_half * hd], BF16, tag="vt")
                    nc.sync.dma_start(
                        vt[:],
                        v_d[bb, :, hf * kt_half * hd:(hf + 1) * kt_half * hd],
                    )
                    for g in range(kt_half // GRP):
                        sc_ps = ps.tile([128, GRP * rows], F32, tag="sc")
                        for u in range(GRP):
                            tt = g * GRP + u
                            nc.tensor.matmul(
                                sc_ps[:, u * rows:(u + 1) * rows],
                                kt[:, tt * 128:(tt + 1) * 128], qT_b,
                                start=(u == 0), stop=(u == GRP - 1),
                            )
                        ex = exp_p.tile([128, GRP * rows], BF16, tag="ex")
                        nc.scalar.activation(ex[:], sc_ps[:], AF.Exp)
                        red = acc if (hf == 0 and g == 0) else acc_p.tile(
                            [128, rows], F32, tag="red", name="red")
                        nc.vector.tensor_reduce(
                            red[:],
                            ex[:].rearrange("p (u q) -> p q u", u=GRP),
                            axis=mybir.AxisListType.X, op=mybir.AluOpType.add,
                        )
                        if red is not acc:
                            nc.vector.tensor_add(acc[:], acc[:], red[:])
                        for u in range(GRP):
                            tt = g * GRP + u
                            t = hf * kt_half + tt
                            nc.tensor.matmul(
                                attn_ps[:], vt[:, tt * hd:(tt + 1) * hd],
                                ex[:, u * rows:(u + 1) * rows],
                                start=(t == 0), stop=False, skip_group_check=True,
                            )
                # fresh keys (the only masked block)
                scn_ps = ps.tile([s, rows], F32, tag="sc")
                nc.tensor.matmul(
                    scn_ps[:], kT_new[:, bb * s:(bb + 1) * s], qT_b32,
                    start=True, stop=True,
                )
                exn = exp_p.tile([s, rows], F32, tag="exn")
                nc.scalar.activation(exn[:], scn_ps[:], AF.Exp)
                nc.vector.tensor_mul(exn[:], exn[:], maskT[:])
                nc.vector.tensor_add(acc[0:s, :], acc[0:s, :], exn[:])
                nc.tensor.matmul(
                    attn_ps[:], v_new[bb][:], exn[:],
                    start=False, stop=True, skip_group_check=True,
                )
                # denominator: reduce acc over partitions, broadcast reciprocal
                dsum_ps = ps.tile([1, rows], F32, tag="B")
                nc.tensor.matmul(dsum_ps[:], ones_col[:], acc[:], start=True, stop=True)
                rden = den_p.tile([1, rows], F32, tag="rden")
                nc.vector.reciprocal(rden[:], dsum_ps[:])
                bc_ps = ps.tile([128, rows], F32, tag="B")
                nc.tensor.matmul(bc_ps[:], ones_row[:], rden[:], start=True, stop=True)
                rdenb = den_p.tile([128, rows], F32, tag="rdenb")
                nc.scalar.copy(rdenb[:], bc_ps[:])
                # normalize + scatter (h,t) -> (h, b, t)
                dst = attnT_sb[:].rearrange("p (j bb t) -> p j bb t", j=hq, bb=b)[
                    :, :, bb, :
                ]
                nc.vector.tensor_mul(
                    dst,
                    attn_ps[:].rearrange("p (j t) -> p j t", j=hq),
                    rdenb[:].rearrange("p (j t) -> p j t", j=hq),
                )

            # ---- o_proj: out[tok, h] = sum_j attnT_j.T @ wo_j ----
            for nt in range(h // 512):
                o_ps = ps.tile([tok, 512], F32, tag="A")
                for j in range(hq):
                    wo_t = wo_p.tile([128, 512], BF16, tag="wo")
                    nc.sync.dma_start(
                        wo_t[:],
                        wo_d.rearrange("(j p) m -> j p m", p=128)[
                            j, :, nt * 512:(nt + 1) * 512
                        ],
                    )
                    nc.tensor.matmul(
                        o_ps[:], attnT_sb[:, j * tok:(j + 1) * tok], wo_t[:],
                        start=(j == 0), stop=(j == hq - 1),
                    )
                o_sb = wo_p.tile([tok, 512], F32, tag="osb", bufs=3)
                nc.scalar.copy(o_sb[:], o_ps[:])
                nc.sync.dma_start(out_d[:, nt * 512:(nt + 1) * 512], o_sb[:])

    nc.compile()
    return nc


_NC_CACHE = {}


def _get_nc(key=(B, S, H, HQ, HD, PAST)):
    if key not in _NC_CACHE:
        _NC_CACHE[key] = build_nc(*key)
    return _NC_CACHE[key]


def make_in_maps(hidden_states, k_cache, v_cache, Wq, Wk, Wv, Wo, position_ids):
    """Host-side shard + layout prep: one input dict per core."""
    hiddenT = np.ascontiguousarray(
        hidden_states.reshape(TOK, H).T.astype(np.float32)
    ).astype(ml_dtypes.bfloat16)
    # RoPE tables in [d, (b, t)] layout, duplicated freq block (half-split rope)
    inv_freq = (1.0 / (ROPE_BASE ** (np.arange(0, HD, 2, dtype=np.float64) / HD)))
    ang = position_ids.astype(np.float64).reshape(-1)[None, :] * np.concatenate(
        [inv_freq, inv_freq]
    )[:, None]                                           # [hd, tok]
    cosT = np.cos(ang).astype(np.float32)
    sinT = np.sin(ang).astype(np.float32)
    nsinT = (-sinT).copy()
    # mask over fresh keys: maskT[j, (h, t)] = 1 if j <= t (bottom-right causal)
    jj = np.arange(S)[:, None]
    tt = np.tile(np.arange(S)[None, :], (1, HQ)).reshape(1, ROWS)
    maskT = (jj <= tt).astype(np.float32)

    in_maps = []
    for c in range(NCORES):
        q0 = c * HQ * HD
        in_maps.append({
            "hiddenT": hiddenT,
            "wq": np.ascontiguousarray(
                (Wq[:, q0:q0 + HQ * HD] * SCALE).astype(np.float32)
            ).astype(ml_dtypes.bfloat16),
            "wkv": np.ascontiguousarray(
                np.concatenate(
                    [Wk[:, c * HD:(c + 1) * HD], Wv[:, c * HD:(c + 1) * HD]], axis=1
                ), dtype=np.float32).astype(ml_dtypes.bfloat16),
            "wo": np.ascontiguousarray(
                Wo[q0:q0 + HQ * HD, :].astype(np.float32)
            ).astype(ml_dtypes.bfloat16),
            "kT": np.ascontiguousarray(
                k_cache[:, :, c, :].transpose(0, 2, 1)).astype(ml_dtypes.bfloat16),
            # pre-permuted to the sbuf tile layout: v_r[b, p, tt*HD+d] =
            # v[b, tt*128+p, d] -> fully contiguous 8KB DMA rows
            "v": np.ascontiguousarray(
                v_cache[:, :, c, :].reshape(B, PAST // 128, 128, HD)
                .transpose(0, 2, 1, 3).reshape(B, 128, PAST)
            ).astype(ml_dtypes.bfloat16),
            "cosT": cosT, "sinT": sinT, "nsinT": nsinT, "maskT": maskT,
        })
    return in_maps


def kernel(hidden_states, k_cache, v_cache, Wq, Wk, Wv, Wo, position_ids):
    hidden_states = np.asarray(hidden_states)
    nc = _get_nc()
    in_maps = make_in_maps(
        np.asarray(hidden_states), np.asarray(k_cache), np.asarray(v_cache),
        np.asarray(Wq), np.asarray(Wk), np.asarray(Wv), np.asarray(Wo),
        np.asarray(position_ids),
    )
    res = run_bass_kernel_spmd(nc, in_maps, list(range(NCORES)))
    out = np.zeros((TOK, H), np.float32)
    for c in range(NCORES):
        out += res.results[c]["out_p"]
    return out.reshape(B, S, H)



# revision 4
# speedup vs baseline: 1.3320x; 1.3320x over previous
"""Tensor-parallel Llama attention (decode, GQA, RoPE, KV-cache) on 8 TRN2 cores.

Sharding: core c owns kv-head c and q-heads 4c..4c+3. Wq/Wk/Wv are sharded
column-wise, Wo row-wise; each core computes a partial o_proj output and the
host sums the 8 partials (the all-reduce).

v2 layout notes (all driven by the serial DMA/HWDGE devices in the timeline
model: ~625ns fixed cost per DMA instruction + 360GB/s aggregate transfer):
  - Every DRAM tensor is host-pre-arranged to the exact SBUF tile layout so
    each load is one large contiguous-elem DMA (>=1KB runs). ~45 DMAs total.
  - All weights live in SBUF for the whole kernel (wq 32KB/part, wkv 16,
    wo 32); only the kv cache streams (1MB tiles, 3-deep per tag).
  - Weights+consts go on the Activation hwdge queue, hT+kv+wo on the SP
    queue, so kv streaming and weight loads interleave at the DMA device
    while staying ordered within each queue (wo after kv by queue order).
  - q projection is computed transposed (qT[j] = Wq_j.T @ hT chunks) so all
    proj matmuls use the full 128-partition output and no PE transposes or
    identity are needed; RoPE reads the qT PSUM tiles directly.
  - Scores stay [kpos, rows] with 8-tile PSUM groups (512-wide exp), the
    attn.V accumulation interleaves one group behind the scores, and the
    softmax denominator is a ones-column matmul + reciprocal broadcast.
  - PSUM budget (8 banks): qt(4): qT heads -> o_proj rotation; B(2):
    kT/v proj -> per-batch attn accumulators; sc(2): score groups, fresh
    scores, dsum, bc.
"""

import numpy as np
import ml_dtypes

import concourse.bass as bass
import concourse.mybir as mybir
import concourse.tile as tile
from concourse import bacc
from concourse.bass_utils import run_bass_kernel_spmd

F32 = mybir.dt.float32
BF16 = mybir.dt.bfloat16
AF = mybir.ActivationFunctionType

# Problem shape (hardcoded per contract)
B, S, H = 4, 16, 4096
NH, NKV, HD = 32, 8, 128
PAST = 8192
ROPE_BASE = 10000.0
NCORES = 8
HQ = NH // NCORES          # q heads per core = 4
TOK = B * S                # 64 tokens
NCH = H // 128             # 32 contraction chunks for projections
ROWS = HQ * S              # 64 (head, token) query rows per batch
SCALE = HD ** -0.5


def build_nc(b=B, s=S, h=H, hq=HQ, hd=HD, past=PAST):
    tok = b * s
    nch = h // 128
    rows = hq * s
    half_kv = past // 2                 # kv streamed in 2 x 1MB tiles per batch
    GRP = 8                             # kpos-tiles per score group (512 cols)
    KPG = GRP * 128                     # kpos per group = 1024
    ngrp = past // KPG                  # 8 groups per batch

    nc = bacc.Bacc("TRN2", target_bir_lowering=False, debug=False)

    # Host-side pre-arranged layouts (see make_in_maps)
    hT_d = nc.dram_tensor("hT", [128, nch * tok], BF16, kind="ExternalInput").ap()
    wq_d = nc.dram_tensor("wq", [128, nch * hq * hd], BF16, kind="ExternalInput").ap()
    wkv_d = nc.dram_tensor("wkv", [128, nch * 2 * hd], BF16, kind="ExternalInput").ap()
    wo_d = nc.dram_tensor("wo", [128, hq * h], BF16, kind="ExternalInput").ap()
    kT_d = nc.dram_tensor("kT", [b, 128, past], BF16, kind="ExternalInput").ap()
    v_d = nc.dram_tensor("v", [b, 128, past], BF16, kind="ExternalInput").ap()
    # const blob: cols 0:64 cosT, 64:128 sinT, 128:192 -sinT, 192:256 mask(rows 0:16)
    blob_d = nc.dram_tensor("blob", [128, 4 * tok], F32, kind="ExternalInput").ap()
    out_d = nc.dram_tensor("out_p", [tok, h], F32, kind="ExternalOutput").ap()

    with tile.TileContext(nc) as tc:
        import contextlib

        with contextlib.ExitStack() as ctx:
            ep = ctx.enter_context
            const_p = ep(tc.tile_pool(name="const", bufs=1))
            w_p = ep(tc.tile_pool(name="w", bufs=1))
            kv_p = ep(tc.tile_pool(name="kv", bufs=3))
            qkv_p = ep(tc.tile_pool(name="qkv", bufs=1))
            rope_p = ep(tc.tile_pool(name="rope", bufs=2))
            exp_p = ep(tc.tile_pool(name="exp", bufs=6))
            acc_p = ep(tc.tile_pool(name="acc", bufs=2))
            den_p = ep(tc.tile_pool(name="den", bufs=2))
            o_p = ep(tc.tile_pool(name="o", bufs=4))
            ps = ep(tc.tile_pool(name="ps", bufs=2, space="PSUM"))

            # ---- consts (scalar/Act hwdge queue) ----
            blob = const_p.tile([128, 4 * tok], F32)
            nc.scalar.dma_start(blob[:], blob_d[:])
            cosT = blob[:, 0:tok]
            sinT = blob[:, tok:2 * tok]
            nsinT = blob[:, 2 * tok:3 * tok]
            maskT = blob[0:s, 3 * tok:3 * tok + rows]
            ones_col = const_p.tile([128, 1], F32)
            nc.vector.memset(ones_col[:], 1.0)
            ones_row = const_p.tile([1, 128], F32)
            nc.vector.memset(ones_row[:], 1.0)

            # ---- weights resident in SBUF; quarters so proj can pipeline ----
            wkv_sb = w_p.tile([128, nch * 2 * hd], BF16, tag="wkv")
            wq_sb = w_p.tile([128, nch * hq * hd], BF16, tag="wq")
            qw = nch * hq * hd // 4     # wq quarter cols (8 chunks each)
            hw_ = nch * 2 * hd // 2     # wkv half cols (16 chunks each)
            nc.scalar.dma_start(wkv_sb[:, 0:hw_], wkv_d[:, 0:hw_])
            nc.scalar.dma_start(wq_sb[:, 0:qw], wq_d[:, 0:qw])
            nc.scalar.dma_start(wq_sb[:, qw:2 * qw], wq_d[:, qw:2 * qw])
            nc.scalar.dma_start(wkv_sb[:, hw_:2 * hw_], wkv_d[:, hw_:2 * hw_])
            nc.scalar.dma_start(wq_sb[:, 2 * qw:3 * qw], wq_d[:, 2 * qw:3 * qw])
            nc.scalar.dma_start(wq_sb[:, 3 * qw:4 * qw], wq_d[:, 3 * qw:4 * qw])

            # ---- hT (sync/SP hwdge queue, ahead of the kv stream) ----
            hT = w_p.tile([128, nch * tok], BF16, tag="hT")
            nc.sync.dma_start(hT[:], hT_d[:])

            # ---- projections, q transposed: qT_ps[j] = Wq_j.T @ h ----
            qT_ps = [
                ps.tile([hd, tok], F32, tag="qt", bufs=4, name=f"qt{j}")
                for j in range(hq)
            ]
            kT_ps = ps.tile([hd, tok], F32, tag="B")
            v_ps = ps.tile([tok, hd], F32, tag="B")
            for c in range(nch):
                rhs_h = hT[:, c * tok:(c + 1) * tok]
                fl = dict(start=(c == 0), stop=(c == nch - 1))
                for j in range(hq):
                    nc.tensor.matmul(
                        qT_ps[j][:],
                        wq_sb[:, c * hq * hd + j * hd:c * hq * hd + (j + 1) * hd],
                        rhs_h, **fl,
                    )
                nc.tensor.matmul(
                    kT_ps[:], wkv_sb[:, c * 2 * hd:c * 2 * hd + hd], rhs_h, **fl
                )
                nc.tensor.matmul(
                    v_ps[:], rhs_h, wkv_sb[:, c * 2 * hd + hd:(c + 1) * 2 * hd], **fl
                )

            # ---- RoPE -> qT_sb [128, (b,hq,s)], kT_new [128, (b,s)] ----
            half = hd // 2
            qT_sb = qkv_p.tile([128, b * rows], F32, tag="qT")
            kT_new = qkv_p.tile([128, tok], F32, tag="kTn")

            def rope(dst, src_ps):
                t1 = rope_p.tile([128, tok], F32, tag="r1")
                nc.vector.tensor_mul(t1[:], src_ps[:], cosT)
                t2 = rope_p.tile([128, tok], F32, tag="r2")
                nc.vector.tensor_mul(
                    t2[0:half, :], src_ps[half:hd, :], nsinT[0:half, :]
                )
                nc.vector.tensor_mul(
                    t2[half:hd, :], src_ps[0:half, :], sinT[half:hd, :]
                )
                nc.vector.tensor_add(dst, t1[:], t2[:])

            for j in range(hq):
                dst = qT_sb[:].rearrange("p (bb j t) -> p bb j t", bb=b, j=hq)[:, :, j, :]
                rope(dst, qT_ps[j][:])
            rope(kT_new[:], kT_ps[:])
            qT_bf = qkv_p.tile([128, b * rows], BF16, tag="qTbf")
            nc.vector.tensor_copy(qT_bf[:], qT_sb[:])

            # fresh v rows per batch at partition base 0 (PE stationary operand)
            v_sb = qkv_p.tile([tok, hd], F32, tag="vsb")
            nc.scalar.copy(v_sb[:], v_ps[:])
            v_new = [
                qkv_p.tile([s, hd], F32, tag=f"vnew{bb}", name=f"vnew{bb}")
                for bb in range(b)
            ]
            for bb in range(b):
                nc.scalar.dma_start(v_new[bb][:], v_sb[bb * s:(bb + 1) * s, :])

            # ---- attention per batch ----
            attnT_sb = qkv_p.tile([128, hq * tok], BF16, tag="attnT")  # (j, b, t)
            for bb in range(b):
                qT_b = qT_bf[:, bb * rows:(bb + 1) * rows]
                qT_b32 = qT_sb[:, bb * rows:(bb + 1) * rows]
                attn_ps = ps.tile([hd, rows], F32, tag="B", name=f"attn{bb}")
                acc = acc_p.tile([128, rows], F32, tag="acc")
                kts, vts = [], []
                for hf in range(2):
                    kt = kv_p.tile([128, half_kv], BF16, tag="kt", name=f"kt{bb}{hf}")
                    nc.sync.dma_start(
                        kt[:], kT_d[bb, :, hf * half_kv:(hf + 1) * half_kv]
                    )
                    kts.append(kt)
                    vt = kv_p.tile([128, half_kv], BF16, tag="vt", name=f"vt{bb}{hf}")
                    nc.sync.dma_start(
                        vt[:], v_d[bb, :, hf * half_kv:(hf + 1) * half_kv]
                    )
                    vts.append(vt)

                exs = [None] * ngrp

                def attn_group(g):
                    vt = vts[g // (ngrp // 2)]
                    off = (g % (ngrp // 2)) * KPG
                    for u in range(GRP):
                        nc.tensor.matmul(
                            attn_ps[:], vt[:, off + u * 128:off + (u + 1) * 128],
                            exs[g][:, u * rows:(u + 1) * rows],
                            start=(g == 0 and u == 0), stop=False,
                            skip_group_check=True,
                        )

                for g in range(ngrp):
                    kt = kts[g // (ngrp // 2)]
                    off = (g % (ngrp // 2)) * KPG
                    sc_ps = ps.tile([128, GRP * rows], F32, tag="sc", name=f"sc{bb}{g}")
                    for u in range(GRP):
                        nc.tensor.matmul(
                            sc_ps[:, u * rows:(u + 1) * rows],
                            kt[:, off + u * 128:off + (u + 1) * 128], qT_b,
                            start=(u == 0), stop=(u == GRP - 1),
                        )
                    ex = exp_p.tile([128, GRP * rows], BF16, tag="ex")
                    nc.scalar.activation(ex[:], sc_ps[:], AF.Exp)
                    exs[g] = ex
                    red = acc if g == 0 else acc_p.tile(
                        [128, rows], F32, tag="red", name="red")
                    nc.vector.tensor_reduce(
                        red[:],
                        ex[:].rearrange("p (u q) -> p q u", u=GRP),
                        axis=mybir.AxisListType.X, op=mybir.AluOpType.add,
                    )
                    if red is not acc:
                        nc.vector.tensor_add(acc[:], acc[:], red[:])
                    if g > 0:
                        attn_group(g - 1)
                attn_group(ngrp - 1)

                # fresh keys (the only masked block)
                scn_ps = ps.tile([s, rows], F32, tag="sc", name=f"scn{bb}")
                nc.tensor.matmul(
                    scn_ps[:], kT_new[:, bb * s:(bb + 1) * s], qT_b32,
                    start=True, stop=True,
                )
                exn = exp_p.tile([s, rows], F32, tag="exn")
                nc.scalar.activation(exn[:], scn_ps[:], AF.Exp)
                nc.vector.tensor_mul(exn[:], exn[:], maskT)
                nc.vector.tensor_add(acc[0:s, :], acc[0:s, :], exn[:])
                nc.tensor.matmul(
                    attn_ps[:], v_new[bb][:], exn[:],
                    start=False, stop=True, skip_group_check=True,
                )
                # denominator: reduce acc over partitions, broadcast reciprocal
                dsum_ps = ps.tile([1, rows], F32, tag="sc", name=f"ds{bb}")
                nc.tensor.matmul(dsum_ps[:], ones_col[:], acc[:], start=True, stop=True)
                rden = den_p.tile([1, rows], F32, tag="rden")
                nc.vector.reciprocal(rden[:], dsum_ps[:])
                bc_ps = ps.tile([128, rows], F32, tag="sc", name=f"bc{bb}")
                nc.tensor.matmul(bc_ps[:], ones_row[:], rden[:], start=True, stop=True)
                rdenb = den_p.tile([128, rows], F32, tag="rdenb")
                nc.scalar.copy(rdenb[:], bc_ps[:])
                dst = attnT_sb[:].rearrange("p (j bb t) -> p j bb t", j=hq, bb=b)[
                    :, :, bb, :
                ]
                nc.vector.tensor_mul(
                    dst,
                    attn_ps[:].rearrange("p (j t) -> p j t", j=hq),
                    rdenb[:].rearrange("p (j t) -> p j t", j=hq),
                )

            # ---- o_proj: wo column-quarters stream on the sync queue (after kv),
            # out[tok, h] accumulated over j per 512-col block ----
            wo_sb = w_p.tile([128, hq * h], BF16, tag="wo")
            wo_v = wo_sb[:].rearrange("p (j m) -> p j m", j=hq)
            wo_dv = wo_d.rearrange("p (j m) -> p j m", j=hq)
            qcols = h // 4
            for q in range(4):
                nc.sync.dma_start(
                    wo_v[:, :, q * qcols:(q + 1) * qcols],
                    wo_dv[:, :, q * qcols:(q + 1) * qcols],
                )
            for nt in range(h // 512):
                o_ps = ps.tile([tok, 512], F32, tag="qt", bufs=4, name=f"o{nt}")
                for j in range(hq):
                    nc.tensor.matmul(
                        o_ps[:], attnT_sb[:, j * tok:(j + 1) * tok],
                        wo_v[:, j, nt * 512:(nt + 1) * 512],
                        start=(j == 0), stop=(j == hq - 1),
                    )
                o_sb = o_p.tile([tok, 512], F32, tag="osb")
                nc.scalar.copy(o_sb[:], o_ps[:])
                # SP queue: keeps Act.SEQ free for the next block's copy
                nc.sync.dma_start(out_d[:, nt * 512:(nt + 1) * 512], o_sb[:])

    nc.compile()
    return nc


_NC_CACHE = {}


def _get_nc(key=(B, S, H, HQ, HD, PAST)):
    if key not in _NC_CACHE:
        _NC_CACHE[key] = build_nc(*key)
    return _NC_CACHE[key]


def make_in_maps(hidden_states, k_cache, v_cache, Wq, Wk, Wv, Wo, position_ids):
    """Host-side shard + layout prep: one input dict per core."""
    bf16 = ml_dtypes.bfloat16
    # hT[p, (c, t)] = hidden[t, c*128+p]
    hT = np.ascontiguousarray(
        hidden_states.reshape(TOK, H).T.astype(np.float32)
        .reshape(NCH, 128, TOK).transpose(1, 0, 2).reshape(128, NCH * TOK)
    ).astype(bf16)
    # RoPE tables in [d, (b, t)] layout, duplicated freq block (half-split rope)
    inv_freq = (1.0 / (ROPE_BASE ** (np.arange(0, HD, 2, dtype=np.float64) / HD)))
    ang = position_ids.astype(np.float64).reshape(-1)[None, :] * np.concatenate(
        [inv_freq, inv_freq]
    )[:, None]                                           # [hd, tok]
    blob = np.zeros((128, 4 * TOK), np.float32)
    blob[:, 0:TOK] = np.cos(ang)
    blob[:, TOK:2 * TOK] = np.sin(ang)
    blob[:, 2 * TOK:3 * TOK] = -np.sin(ang)
    # mask over fresh keys: mask[j, (h, t)] = 1 if j <= t (bottom-right causal)
    jj = np.arange(S)[:, None]
    tt = np.tile(np.arange(S)[None, :], (1, HQ)).reshape(1, ROWS)
    blob[0:S, 3 * TOK:3 * TOK + ROWS] = (jj <= tt).astype(np.float32)

    in_maps = []
    for c in range(NCORES):
        q0 = c * HQ * HD
        # wq[p, (ch, j*hd+d)] = Wq[ch*128+p, q0 + j*hd+d] * SCALE
        wq = np.ascontiguousarray(
            (Wq[:, q0:q0 + HQ * HD] * SCALE).astype(np.float32)
            .reshape(NCH, 128, HQ * HD).transpose(1, 0, 2).reshape(128, -1)
        ).astype(bf16)
        wkv = np.ascontiguousarray(
            np.concatenate(
                [Wk[:, c * HD:(c + 1) * HD], Wv[:, c * HD:(c + 1) * HD]], axis=1
            ).astype(np.float32)
            .reshape(NCH, 128, 2 * HD).transpose(1, 0, 2).reshape(128, -1)
        ).astype(bf16)
        # wo[p, (j, m)] = Wo[q0 + j*128+p, m]
        wo = np.ascontiguousarray(
            Wo[q0:q0 + HQ * HD, :].astype(np.float32)
            .reshape(HQ, 128, H).transpose(1, 0, 2).reshape(128, -1)
        ).astype(bf16)
        in_maps.append({
            "hT": hT,
            "wq": wq,
            "wkv": wkv,
            "wo": wo,
            "kT": np.ascontiguousarray(
                k_cache[:, :, c, :].transpose(0, 2, 1)).astype(bf16),
            # v_r[b, p, tt*HD+d] = v[b, tt*128+p, d] -> contiguous 8KB DMA rows
            "v": np.ascontiguousarray(
                v_cache[:, :, c, :].reshape(B, PAST // 128, 128, HD)
                .transpose(0, 2, 1, 3).reshape(B, 128, PAST)
            ).astype(bf16),
            "blob": blob,
        })
    return in_maps


def kernel(hidden_states, k_cache, v_cache, Wq, Wk, Wv, Wo, position_ids):
    nc = _get_nc()
    in_maps = make_in_maps(
        np.asarray(hidden_states), np.asarray(k_cache), np.asarray(v_cache),
        np.asarray(Wq), np.asarray(Wk), np.asarray(Wv), np.asarray(Wo),
        np.asarray(position_ids),
    )
    res = run_bass_kernel_spmd(nc, in_maps, list(range(NCORES)))
    out = np.zeros((TOK, H), np.float32)
    for c in range(NCORES):
        out += res.results[c]["out_p"]
    return out.reshape(B, S, H)


# revision 10
# speedup vs baseline: 1.4418x; 1.0825x over previous
"""Tensor-parallel Llama attention (decode, GQA, RoPE, KV-cache) on 8 TRN2 cores.

Sharding: core c owns kv-head c and q-heads 4c..4c+3. Wq/Wk/Wv are sharded
column-wise, Wo row-wise; each core computes a partial o_proj output and the
host sums the 8 partials (the all-reduce).

v2 layout notes (all driven by the serial DMA/HWDGE devices in the timeline
model: ~625ns fixed cost per DMA instruction + 360GB/s aggregate transfer):
  - Every DRAM tensor is host-pre-arranged to the exact SBUF tile layout so
    each load is one large contiguous-elem DMA (>=1KB runs). ~45 DMAs total.
  - All weights live in SBUF for the whole kernel (wq 32KB/part, wkv 16,
    wo 32); only the kv cache streams (1MB tiles, 3-deep per tag).
  - Weights+consts go on the Activation hwdge queue, hT+kv+wo on the SP
    queue, so kv streaming and weight loads interleave at the DMA device
    while staying ordered within each queue (wo after kv by queue order).
  - q projection is computed transposed (qT[j] = Wq_j.T @ hT chunks) so all
    proj matmuls use the full 128-partition output and no PE transposes or
    identity are needed; RoPE reads the qT PSUM tiles directly.
  - Scores stay [kpos, rows] with 8-tile PSUM groups (512-wide exp), the
    attn.V accumulation interleaves one group behind the scores, and the
    softmax denominator is a ones-column matmul + reciprocal broadcast.
  - PSUM budget (8 banks): qt(4): qT heads -> o_proj rotation; B(2):
    kT/v proj -> per-batch attn accumulators; sc(2): score groups, fresh
    scores, dsum, bc.
"""

import numpy as np
import ml_dtypes

import concourse.bass as bass
import concourse.mybir as mybir
import concourse.tile as tile
from concourse import bacc
from concourse.bass_utils import run_bass_kernel_spmd

F32 = mybir.dt.float32
BF16 = mybir.dt.bfloat16
AF = mybir.ActivationFunctionType

# Problem shape (hardcoded per contract)
B, S, H = 4, 16, 4096
NH, NKV, HD = 32, 8, 128
PAST = 8192
ROPE_BASE = 10000.0
NCORES = 8
HQ = NH // NCORES          # q heads per core = 4
TOK = B * S                # 64 tokens
NCH = H // 128             # 32 contraction chunks for projections
ROWS = HQ * S              # 64 (head, token) query rows per batch
SCALE = HD ** -0.5


def build_nc(b=B, s=S, h=H, hq=HQ, hd=HD, past=PAST):
    tok = b * s
    nch = h // 128
    rows = hq * s
    half_kv = past // 2                 # kv streamed in 2 x 1MB tiles per batch
    GRP = 8                             # kpos-tiles per score group (512 cols)
    KPG = GRP * 128                     # kpos per group = 1024
    ngrp = past // KPG                  # 8 groups per batch

    nc = bacc.Bacc("TRN2", target_bir_lowering=False, debug=False)

    # Host-side pre-arranged layouts (see make_in_maps)
    hT_d = nc.dram_tensor("hT", [128, nch * tok], BF16, kind="ExternalInput").ap()
    wq_d = nc.dram_tensor("wq", [128, nch * hq * hd], BF16, kind="ExternalInput").ap()
    wkv_d = nc.dram_tensor("wkv", [128, nch * 2 * hd], BF16, kind="ExternalInput").ap()
    wo_d = nc.dram_tensor("wo", [128, hq * h], BF16, kind="ExternalInput").ap()
    kT_d = nc.dram_tensor("kT", [b, 128, past], BF16, kind="ExternalInput").ap()
    v_d = nc.dram_tensor("v", [b, 128, past], BF16, kind="ExternalInput").ap()
    # const blob: cols 0:64 cosT, 64:128 sinT, 128:192 -sinT, 192:256 mask(rows 0:16)
    blob_d = nc.dram_tensor("blob", [128, 4 * tok], F32, kind="ExternalInput").ap()
    out_d = nc.dram_tensor("out_p", [tok, h], F32, kind="ExternalOutput").ap()

    with tile.TileContext(nc) as tc:
        import contextlib

        with contextlib.ExitStack() as ctx:
            ep = ctx.enter_context
            const_p = ep(tc.tile_pool(name="const", bufs=1))
            w_p = ep(tc.tile_pool(name="w", bufs=1))
            kv_p = ep(tc.tile_pool(name="kv", bufs=3))
            qkv_p = ep(tc.tile_pool(name="qkv", bufs=1))
            rope_p = ep(tc.tile_pool(name="rope", bufs=2))
            exp_p = ep(tc.tile_pool(name="exp", bufs=6))
            acc_p = ep(tc.tile_pool(name="acc", bufs=2))
            den_p = ep(tc.tile_pool(name="den", bufs=2))
            o_p = ep(tc.tile_pool(name="o", bufs=8))
            ps = ep(tc.tile_pool(name="ps", bufs=2, space="PSUM"))

            # ---- consts (scalar/Act hwdge queue) ----
            blob = const_p.tile([128, 4 * tok], F32)
            nc.scalar.dma_start(blob[:], blob_d[:])
            cosT = blob[:, 0:tok]
            sinT = blob[:, tok:2 * tok]
            nsinT = blob[:, 2 * tok:3 * tok]
            maskT = blob[0:s, 3 * tok:3 * tok + rows]
            ones_col = const_p.tile([128, 1], F32)
            nc.vector.memset(ones_col[:], 1.0)
            ones_row = const_p.tile([1, 128], F32)
            nc.vector.memset(ones_row[:], 1.0)

            # ---- weights resident in SBUF; separate tiles per ~1MB DMA so
            # consumers only wait for their own quarter ----
            qw = nch * hq * hd // 4     # wq quarter cols (8 chunks each)
            hw_ = nch * 2 * hd // 2     # wkv half cols (16 chunks each)
            wq_t = [
                w_p.tile([128, qw], BF16, tag=f"wq{i}", name=f"wq{i}")
                for i in range(4)
            ]
            wkv_t = [
                w_p.tile([128, hw_], BF16, tag=f"wkv{i}", name=f"wkv{i}")
                for i in range(2)
            ]
            # All big loads share the SP queue in exact consumption order:
            # the scheduler keeps program order among ready DMAs, so weights
            # are guaranteed to precede the kv stream at the DMA device.
            hT = w_p.tile([128, nch * tok], BF16, tag="hT")
            nc.sync.dma_start(hT[:], hT_d[:])
            nc.sync.dma_start(wkv_t[0][:], wkv_d[:, 0:hw_])
            nc.sync.dma_start(wq_t[0][:], wq_d[:, 0:qw])
            nc.sync.dma_start(wq_t[1][:], wq_d[:, qw:2 * qw])
            nc.sync.dma_start(wkv_t[1][:], wkv_d[:, hw_:2 * hw_])
            nc.sync.dma_start(wq_t[2][:], wq_d[:, 2 * qw:3 * qw])
            nc.sync.dma_start(wq_t[3][:], wq_d[:, 3 * qw:4 * qw])

            # ---- projections, q transposed: qT_ps[j] = Wq_j.T @ h ----
            qT_ps = [
                ps.tile([hd, tok], F32, tag="qt", bufs=4, name=f"qt{j}")
                for j in range(hq)
            ]
            kT_ps = ps.tile([hd, tok], F32, tag="B")
            v_ps = ps.tile([tok, hd], F32, tag="B")
            for c in range(nch):
                rhs_h = hT[:, c * tok:(c + 1) * tok]
                fl = dict(start=(c == 0), stop=(c == nch - 1))
                wq_q = wq_t[c // 8]
                cq = (c % 8) * hq * hd
                wkv_h = wkv_t[c // 16]
                ch = (c % 16) * 2 * hd
                for j in range(hq):
                    nc.tensor.matmul(
                        qT_ps[j][:],
                        wq_q[:, cq + j * hd:cq + (j + 1) * hd],
                        rhs_h, **fl,
                    )
                nc.tensor.matmul(
                    kT_ps[:], wkv_h[:, ch:ch + hd], rhs_h, **fl
                )
                nc.tensor.matmul(
                    v_ps[:], rhs_h, wkv_h[:, ch + hd:ch + 2 * hd], **fl
                )

            # ---- RoPE -> qT_sb [128, (b,hq,s)], kT_new [128, (b,s)] ----
            half = hd // 2
            qT_sb = qkv_p.tile([128, b * rows], F32, tag="qT")
            kT_new = qkv_p.tile([128, tok], F32, tag="kTn")

            def rope(dst, src_ps):
                t1 = rope_p.tile([128, tok], F32, tag="r1")
                nc.vector.tensor_mul(t1[:], src_ps[:], cosT)
                t2 = rope_p.tile([128, tok], F32, tag="r2")
                nc.vector.tensor_mul(
                    t2[0:half, :], src_ps[half:hd, :], nsinT[0:half, :]
                )
                nc.vector.tensor_mul(
                    t2[half:hd, :], src_ps[0:half, :], sinT[half:hd, :]
                )
                nc.vector.tensor_add(dst, t1[:], t2[:])

            for j in range(hq):
                dst = qT_sb[:].rearrange("p (bb j t) -> p bb j t", bb=b, j=hq)[:, :, j, :]
                rope(dst, qT_ps[j][:])
            rope(kT_new[:], kT_ps[:])
            qT_bf = qkv_p.tile([128, b * rows], BF16, tag="qTbf")
            nc.vector.tensor_copy(qT_bf[:], qT_sb[:])

            # fresh v rows per batch at partition base 0 (PE stationary operand)
            v_sb = qkv_p.tile([tok, hd], F32, tag="vsb")
            nc.scalar.copy(v_sb[:], v_ps[:])
            v_new = [
                qkv_p.tile([s, hd], F32, tag=f"vnew{bb}", name=f"vnew{bb}")
                for bb in range(b)
            ]
            for bb in range(b):
                nc.scalar.dma_start(v_new[bb][:], v_sb[bb * s:(bb + 1) * s, :])

            # ---- attention per batch ----
            attnT_sb = qkv_p.tile([128, hq * tok], BF16, tag="attnT")  # (j, b, t)
            for bb in range(b):
                qT_b = qT_bf[:, bb * rows:(bb + 1) * rows]
                qT_b32 = qT_sb[:, bb * rows:(bb + 1) * rows]
                attn_ps = ps.tile([hd, rows], F32, tag="B", name=f"attn{bb}")
                acc = acc_p.tile([128, rows], F32, tag="acc")
                kts, vts = [], []
                for hf in range(2):
                    kt = kv_p.tile([128, half_kv], BF16, tag="kt", name=f"kt{bb}{hf}")
                    nc.sync.dma_start(
                        kt[:], kT_d[bb, :, hf * half_kv:(hf + 1) * half_kv]
                    )
                    kts.append(kt)
                    vt = kv_p.tile([128, half_kv], BF16, tag="vt", name=f"vt{bb}{hf}")
                    nc.sync.dma_start(
                        vt[:], v_d[bb, :, hf * half_kv:(hf + 1) * half_kv]
                    )
                    vts.append(vt)

                exs = [None] * ngrp

                def attn_group(g):
                    vt = vts[g // (ngrp // 2)]
                    off = (g % (ngrp // 2)) * KPG
                    for u in range(GRP):
                        nc.tensor.matmul(
                            attn_ps[:], vt[:, off + u * 128:off + (u + 1) * 128],
                            exs[g][:, u * rows:(u + 1) * rows],
                            start=(g == 0 and u == 0), stop=False,
                            skip_group_check=True,
                        )

                for g in range(ngrp):
                    kt = kts[g // (ngrp // 2)]
                    off = (g % (ngrp // 2)) * KPG
                    sc_ps = ps.tile([128, GRP * rows], F32, tag="sc", name=f"sc{bb}{g}")
                    for u in range(GRP):
                        nc.tensor.matmul(
                            sc_ps[:, u * rows:(u + 1) * rows],
                            kt[:, off + u * 128:off + (u + 1) * 128], qT_b,
                            start=(u == 0), stop=(u == GRP - 1),
                        )
                    ex = exp_p.tile([128, GRP * rows], BF16, tag="ex")
                    nc.scalar.activation(ex[:], sc_ps[:], AF.Exp)
                    exs[g] = ex
                    red = acc if g == 0 else acc_p.tile(
                        [128, rows], F32, tag="red", name="red")
                    nc.vector.tensor_reduce(
                        red[:],
                        ex[:].rearrange("p (u q) -> p q u", u=GRP),
                        axis=mybir.AxisListType.X, op=mybir.AluOpType.add,
                    )
                    if red is not acc:
                        nc.vector.tensor_add(acc[:], acc[:], red[:])
                    if g > 0:
                        attn_group(g - 1)
                attn_group(ngrp - 1)

                # fresh keys (the only masked block)
                scn_ps = ps.tile([s, rows], F32, tag="sc", name=f"scn{bb}")
                nc.tensor.matmul(
                    scn_ps[:], kT_new[:, bb * s:(bb + 1) * s], qT_b32,
                    start=True, stop=True,
                )
                exn = exp_p.tile([s, rows], F32, tag="exn")
                nc.scalar.activation(exn[:], scn_ps[:], AF.Exp)
                nc.vector.tensor_mul(exn[:], exn[:], maskT)
                nc.vector.tensor_add(acc[0:s, :], acc[0:s, :], exn[:])
                nc.tensor.matmul(
                    attn_ps[:], v_new[bb][:], exn[:],
                    start=False, stop=True, skip_group_check=True,
                )
                # denominator: reduce acc over partitions, broadcast reciprocal
                dsum_ps = ps.tile([1, rows], F32, tag="sc", name=f"ds{bb}")
                nc.tensor.matmul(dsum_ps[:], ones_col[:], acc[:], start=True, stop=True)
                rden = den_p.tile([1, rows], F32, tag="rden")
                nc.vector.reciprocal(rden[:], dsum_ps[:])
                bc_ps = ps.tile([128, rows], F32, tag="sc", name=f"bc{bb}")
                nc.tensor.matmul(bc_ps[:], ones_row[:], rden[:], start=True, stop=True)
                rdenb = den_p.tile([128, rows], F32, tag="rdenb")
                nc.scalar.copy(rdenb[:], bc_ps[:])
                dst = attnT_sb[:].rearrange("p (j bb t) -> p j bb t", j=hq, bb=b)[
                    :, :, bb, :
                ]
                nc.vector.tensor_mul(
                    dst,
                    attn_ps[:].rearrange("p (j t) -> p j t", j=hq),
                    rdenb[:].rearrange("p (j t) -> p j t", j=hq),
                )

            # ---- o_proj: wo column-quarters stream on the sync queue (after kv),
            # out[tok, h] accumulated over j per 512-col block ----
            wo_dv = wo_d.rearrange("p (j m) -> p j m", j=hq)
            qcols = h // 4
            wo_t = []
            for q in range(4):
                wt = w_p.tile([128, hq * qcols], BF16, tag=f"wo{q}", name=f"wo{q}")
                nc.sync.dma_start(
                    wt[:].rearrange("p (j m) -> p j m", j=hq),
                    wo_dv[:, :, q * qcols:(q + 1) * qcols],
                )
                wo_t.append(wt)
            for nt in range(h // 512):
                o_ps = ps.tile([tok, 512], F32, tag="qt", bufs=4, name=f"o{nt}")
                wt = wo_t[nt // 2][:].rearrange("p (j m) -> p j m", j=hq)
                mo = (nt % 2) * 512
                for j in range(hq):
                    nc.tensor.matmul(
                        o_ps[:], attnT_sb[:, j * tok:(j + 1) * tok],
                        wt[:, j, mo:mo + 512],
                        start=(j == 0), stop=(j == hq - 1),
                    )
                o_sb = o_p.tile([tok, 512], F32, tag="osb")
                nc.scalar.copy(o_sb[:], o_ps[:])
                # SP queue: keeps Act.SEQ free for the next block's copy
                nc.sync.dma_start(out_d[:, nt * 512:(nt + 1) * 512], o_sb[:])

    nc.compile()
    return nc


_NC_CACHE = {}


def _get_nc(key=(B, S, H, HQ, HD, PAST)):
    if key not in _NC_CACHE:
        _NC_CACHE[key] = build_nc(*key)
    return _NC_CACHE[key]


def make_in_maps(hidden_states, k_cache, v_cache, Wq, Wk, Wv, Wo, position_ids):
    """Host-side shard + layout prep: one input dict per core."""
    bf16 = ml_dtypes.bfloat16
    # hT[p, (c, t)] = hidden[t, c*128+p]
    hT = np.ascontiguousarray(
        hidden_states.reshape(TOK, H).T.astype(np.float32)
        .reshape(NCH, 128, TOK).transpose(1, 0, 2).reshape(128, NCH * TOK)
    ).astype(bf16)
    # RoPE tables in [d, (b, t)] layout, duplicated freq block (half-split rope)
    inv_freq = (1.0 / (ROPE_BASE ** (np.arange(0, HD, 2, dtype=np.float64) / HD)))
    ang = position_ids.astype(np.float64).reshape(-1)[None, :] * np.concatenate(
        [inv_freq, inv_freq]
    )[:, None]                                           # [hd, tok]
    blob = np.zeros((128, 4 * TOK), np.float32)
    blob[:, 0:TOK] = np.cos(ang)
    blob[:, TOK:2 * TOK] = np.sin(ang)
    blob[:, 2 * TOK:3 * TOK] = -np.sin(ang)
    # mask over fresh keys: mask[j, (h, t)] = 1 if j <= t (bottom-right causal)
    jj = np.arange(S)[:, None]
    tt = np.tile(np.arange(S)[None, :], (1, HQ)).reshape(1, ROWS)
    blob[0:S, 3 * TOK:3 * TOK + ROWS] = (jj <= tt).astype(np.float32)

    in_maps = []
    for c in range(NCORES):
        q0 = c * HQ * HD
        # wq[p, (ch, j*hd+d)] = Wq[ch*128+p, q0 + j*hd+d] * SCALE
        wq = np.ascontiguousarray(
            (Wq[:, q0:q0 + HQ * HD] * SCALE).astype(np.float32)
            .reshape(NCH, 128, HQ * HD).transpose(1, 0, 2).reshape(128, -1)
        ).astype(bf16)
        wkv = np.ascontiguousarray(
            np.concatenate(
                [Wk[:, c * HD:(c + 1) * HD], Wv[:, c * HD:(c + 1) * HD]], axis=1
            ).astype(np.float32)
            .reshape(NCH, 128, 2 * HD).transpose(1, 0, 2).reshape(128, -1)
        ).astype(bf16)
        # wo[p, (j, m)] = Wo[q0 + j*128+p, m]
        wo = np.ascontiguousarray(
            Wo[q0:q0 + HQ * HD, :].astype(np.float32)
            .reshape(HQ, 128, H).transpose(1, 0, 2).reshape(128, -1)
        ).astype(bf16)
        in_maps.append({
            "hT": hT,
            "wq": wq,
            "wkv": wkv,
            "wo": wo,
            "kT": np.ascontiguousarray(
                k_cache[:, :, c, :].transpose(0, 2, 1)).astype(bf16),
            # v_r[b, p, tt*HD+d] = v[b, tt*128+p, d] -> contiguous 8KB DMA rows
            "v": np.ascontiguousarray(
                v_cache[:, :, c, :].reshape(B, PAST // 128, 128, HD)
                .transpose(0, 2, 1, 3).reshape(B, 128, PAST)
            ).astype(bf16),
            "blob": blob,
        })
    return in_maps


def kernel(hidden_states, k_cache, v_cache, Wq, Wk, Wv, Wo, position_ids):
    nc = _get_nc()
    in_maps = make_in_maps(
        np.asarray(hidden_states), np.asarray(k_cache), np.asarray(v_cache),
        np.asarray(Wq), np.asarray(Wk), np.asarray(Wv), np.asarray(Wo),
        np.asarray(position_ids),
    )
    res = run_bass_kernel_spmd(nc, in_maps, list(range(NCORES)))
    out = np.zeros((TOK, H), np.float32)
    for c in range(NCORES):
        out += res.results[c]["out_p"]
    return out.reshape(B, S, H)


# revision 14
# speedup vs baseline: 1.4560x; 1.0098x over previous
"""Tensor-parallel Llama attention (decode, GQA, RoPE, KV-cache) on 8 TRN2 cores.

Sharding: core c owns kv-head c and q-heads 4c..4c+3. Wq/Wk/Wv are sharded
column-wise, Wo row-wise; each core computes a partial o_proj output and the
host sums the 8 partials (the all-reduce).

v2 layout notes (all driven by the serial DMA/HWDGE devices in the timeline
model: ~625ns fixed cost per DMA instruction + 360GB/s aggregate transfer):
  - Every DRAM tensor is host-pre-arranged to the exact SBUF tile layout so
    each load is one large contiguous-elem DMA (>=1KB runs). ~45 DMAs total.
  - All weights live in SBUF for the whole kernel (wq 32KB/part, wkv 16,
    wo 32); only the kv cache streams (1MB tiles, 3-deep per tag).
  - Weights+consts go on the Activation hwdge queue, hT+kv+wo on the SP
    queue, so kv streaming and weight loads interleave at the DMA device
    while staying ordered within each queue (wo after kv by queue order).
  - q projection is computed transposed (qT[j] = Wq_j.T @ hT chunks) so all
    proj matmuls use the full 128-partition output and no PE transposes or
    identity are needed; RoPE reads the qT PSUM tiles directly.
  - Scores stay [kpos, rows] with 8-tile PSUM groups (512-wide exp), the
    attn.V accumulation interleaves one group behind the scores, and the
    softmax denominator is a ones-column matmul + reciprocal broadcast.
  - PSUM budget (8 banks): qt(4): qT heads -> o_proj rotation; B(2):
    kT/v proj -> per-batch attn accumulators; sc(2): score groups, fresh
    scores, dsum, bc.
"""

import numpy as np
import ml_dtypes

import concourse.bass as bass
import concourse.mybir as mybir
import concourse.tile as tile
from concourse import bacc
from concourse.bass_utils import run_bass_kernel_spmd

F32 = mybir.dt.float32
BF16 = mybir.dt.bfloat16
AF = mybir.ActivationFunctionType

# Problem shape (hardcoded per contract)
B, S, H = 4, 16, 4096
NH, NKV, HD = 32, 8, 128
PAST = 8192
ROPE_BASE = 10000.0
NCORES = 8
HQ = NH // NCORES          # q heads per core = 4
TOK = B * S                # 64 tokens
NCH = H // 128             # 32 contraction chunks for projections
ROWS = HQ * S              # 64 (head, token) query rows per batch
SCALE = HD ** -0.5


def build_nc(b=B, s=S, h=H, hq=HQ, hd=HD, past=PAST):
    tok = b * s
    nch = h // 128
    rows = hq * s
    half_kv = past // 2                 # kv streamed in 2 x 1MB tiles per batch
    GRP = 8                             # kpos-tiles per score group (512 cols)
    KPG = GRP * 128                     # kpos per group = 1024
    ngrp = past // KPG                  # 8 groups per batch

    nc = bacc.Bacc("TRN2", target_bir_lowering=False, debug=False)

    # Host-side pre-arranged layouts (see make_in_maps)
    hT_d = nc.dram_tensor("hT", [128, nch * tok], BF16, kind="ExternalInput").ap()
    wq_d = nc.dram_tensor("wq", [128, nch * hq * hd], BF16, kind="ExternalInput").ap()
    wkv_d = nc.dram_tensor("wkv", [128, nch * 2 * hd], BF16, kind="ExternalInput").ap()
    wo_d = nc.dram_tensor("wo", [128, hq * h], BF16, kind="ExternalInput").ap()
    kT_d = nc.dram_tensor("kT", [b, 128, past], BF16, kind="ExternalInput").ap()
    v_d = nc.dram_tensor("v", [b, 128, past], BF16, kind="ExternalInput").ap()
    # const blob: cols 0:64 cosT, 64:128 sinT, 128:192 -sinT, 192:256 mask(rows 0:16)
    blob_d = nc.dram_tensor("blob", [128, 4 * tok], F32, kind="ExternalInput").ap()
    out_d = nc.dram_tensor("out_p", [tok, h], BF16, kind="ExternalOutput").ap()

    with tile.TileContext(nc) as tc:
        import contextlib

        with contextlib.ExitStack() as ctx:
            ep = ctx.enter_context
            const_p = ep(tc.tile_pool(name="const", bufs=1))
            w_p = ep(tc.tile_pool(name="w", bufs=1))
            kv_p = ep(tc.tile_pool(name="kv", bufs=3))
            qkv_p = ep(tc.tile_pool(name="qkv", bufs=1))
            rope_p = ep(tc.tile_pool(name="rope", bufs=2))
            exp_p = ep(tc.tile_pool(name="exp", bufs=6))
            acc_p = ep(tc.tile_pool(name="acc", bufs=2))
            den_p = ep(tc.tile_pool(name="den", bufs=2))
            o_p = ep(tc.tile_pool(name="o", bufs=8))
            ps = ep(tc.tile_pool(name="ps", bufs=2, space="PSUM"))

            # ---- consts (scalar/Act hwdge queue) ----
            blob = const_p.tile([128, 4 * tok], F32)
            nc.scalar.dma_start(blob[:], blob_d[:])
            cosT = blob[:, 0:tok]
            sinT = blob[:, tok:2 * tok]
            nsinT = blob[:, 2 * tok:3 * tok]
            maskT = blob[0:s, 3 * tok:3 * tok + rows]
            ones_col = const_p.tile([128, 1], F32)
            nc.vector.memset(ones_col[:], 1.0)
            ones_row = const_p.tile([1, 128], F32)
            nc.vector.memset(ones_row[:], 1.0)

            # ---- weights resident in SBUF; separate tiles per ~1MB DMA so
            # consumers only wait for their own quarter ----
            qw = nch * hq * hd // 4     # wq quarter cols (8 chunks each)
            hw_ = nch * 2 * hd // 2     # wkv half cols (16 chunks each)
            wq_t = [
                w_p.tile([128, qw], BF16, tag=f"wq{i}", name=f"wq{i}")
                for i in range(4)
            ]
            wkv_t = [
                w_p.tile([128, hw_], BF16, tag=f"wkv{i}", name=f"wkv{i}")
                for i in range(2)
            ]
            # All big loads share the SP queue in exact consumption order:
            # the scheduler keeps program order among ready DMAs, so weights
            # are guaranteed to precede the kv stream at the DMA device.
            hT = w_p.tile([128, nch * tok], BF16, tag="hT")
            nc.sync.dma_start(hT[:], hT_d[:])
            nc.sync.dma_start(wkv_t[0][:], wkv_d[:, 0:hw_])
            nc.sync.dma_start(wq_t[0][:], wq_d[:, 0:qw])
            nc.sync.dma_start(wq_t[1][:], wq_d[:, qw:2 * qw])
            nc.sync.dma_start(wkv_t[1][:], wkv_d[:, hw_:2 * hw_])
            nc.sync.dma_start(wq_t[2][:], wq_d[:, 2 * qw:3 * qw])
            nc.sync.dma_start(wq_t[3][:], wq_d[:, 3 * qw:4 * qw])

            # ---- projections, q transposed: qT_ps[j] = Wq_j.T @ h ----
            qT_ps = [
                ps.tile([hd, tok], F32, tag="qt", bufs=4, name=f"qt{j}")
                for j in range(hq)
            ]
            kT_ps = ps.tile([hd, tok], F32, tag="B")
            v_ps = ps.tile([tok, hd], F32, tag="B")
            for c in range(nch):
                rhs_h = hT[:, c * tok:(c + 1) * tok]
                fl = dict(start=(c == 0), stop=(c == nch - 1))
                wq_q = wq_t[c // 8]
                cq = (c % 8) * hq * hd
                wkv_h = wkv_t[c // 16]
                ch = (c % 16) * 2 * hd
                for j in range(hq):
                    nc.tensor.matmul(
                        qT_ps[j][:],
                        wq_q[:, cq + j * hd:cq + (j + 1) * hd],
                        rhs_h, **fl,
                    )
                nc.tensor.matmul(
                    kT_ps[:], wkv_h[:, ch:ch + hd], rhs_h, **fl
                )
                nc.tensor.matmul(
                    v_ps[:], rhs_h, wkv_h[:, ch + hd:ch + 2 * hd], **fl
                )

            # ---- RoPE -> qT_sb [128, (b,hq,s)], kT_new [128, (b,s)] ----
            half = hd // 2
            qT_sb = qkv_p.tile([128, b * rows], F32, tag="qT")
            kT_new = qkv_p.tile([128, tok], F32, tag="kTn")

            def rope(dst, src_ps):
                t1 = rope_p.tile([128, tok], F32, tag="r1")
                nc.vector.tensor_mul(t1[:], src_ps[:], cosT)
                t2 = rope_p.tile([128, tok], F32, tag="r2")
                nc.vector.tensor_mul(
                    t2[0:half, :], src_ps[half:hd, :], nsinT[0:half, :]
                )
                nc.vector.tensor_mul(
                    t2[half:hd, :], src_ps[0:half, :], sinT[half:hd, :]
                )
                nc.vector.tensor_add(dst, t1[:], t2[:])

            for j in range(hq):
                dst = qT_sb[:].rearrange("p (bb j t) -> p bb j t", bb=b, j=hq)[:, :, j, :]
                rope(dst, qT_ps[j][:])
            rope(kT_new[:], kT_ps[:])
            qT_bf = qkv_p.tile([128, b * rows], BF16, tag="qTbf")
            nc.vector.tensor_copy(qT_bf[:], qT_sb[:])

            # fresh v rows per batch at partition base 0 (PE stationary operand)
            v_sb = qkv_p.tile([tok, hd], F32, tag="vsb")
            nc.scalar.copy(v_sb[:], v_ps[:])
            v_new = [
                qkv_p.tile([s, hd], F32, tag=f"vnew{bb}", name=f"vnew{bb}")
                for bb in range(b)
            ]
            for bb in range(b):
                nc.scalar.dma_start(v_new[bb][:], v_sb[bb * s:(bb + 1) * s, :])

            # ---- attention per batch ----
            attnT_sb = qkv_p.tile([128, hq * tok], BF16, tag="attnT")  # (j, b, t)
            for bb in range(b):
                qT_b = qT_bf[:, bb * rows:(bb + 1) * rows]
                qT_b32 = qT_sb[:, bb * rows:(bb + 1) * rows]
                attn_ps = ps.tile([hd, rows], F32, tag="B", name=f"attn{bb}")
                acc = acc_p.tile([128, rows], F32, tag="acc")
                kts, vts = [], []
                for hf in range(2):
                    kt = kv_p.tile([128, half_kv], BF16, tag="kt", name=f"kt{bb}{hf}")
                    nc.sync.dma_start(
                        kt[:], kT_d[bb, :, hf * half_kv:(hf + 1) * half_kv]
                    )
                    kts.append(kt)
                    vt = kv_p.tile([128, half_kv], BF16, tag="vt", name=f"vt{bb}{hf}")
                    nc.sync.dma_start(
                        vt[:], v_d[bb, :, hf * half_kv:(hf + 1) * half_kv]
                    )
                    vts.append(vt)

                exs = [None] * ngrp

                def attn_group(g):
                    vt = vts[g // (ngrp // 2)]
                    off = (g % (ngrp // 2)) * KPG
                    for u in range(GRP):
                        nc.tensor.matmul(
                            attn_ps[:], vt[:, off + u * 128:off + (u + 1) * 128],
                            exs[g][:, u * rows:(u + 1) * rows],
                            start=(g == 0 and u == 0), stop=False,
                            skip_group_check=True,
                        )

                for g in range(ngrp):
                    kt = kts[g // (ngrp // 2)]
                    off = (g % (ngrp // 2)) * KPG
                    sc_ps = ps.tile([128, GRP * rows], F32, tag="sc", name=f"sc{bb}{g}")
                    for u in range(GRP):
                        nc.tensor.matmul(
                            sc_ps[:, u * rows:(u + 1) * rows],
                            kt[:, off + u * 128:off + (u + 1) * 128], qT_b,
                            start=(u == 0), stop=(u == GRP - 1),
                        )
                    ex = exp_p.tile([128, GRP * rows], BF16, tag="ex")
                    nc.scalar.activation(ex[:], sc_ps[:], AF.Exp)
                    exs[g] = ex
                    red = acc if g == 0 else acc_p.tile(
                        [128, rows], F32, tag="red", name="red")
                    nc.vector.tensor_reduce(
                        red[:],
                        ex[:].rearrange("p (u q) -> p q u", u=GRP),
                        axis=mybir.AxisListType.X, op=mybir.AluOpType.add,
                    )
                    if red is not acc:
                        nc.vector.tensor_add(acc[:], acc[:], red[:])
                    if g > 0:
                        attn_group(g - 1)
                attn_group(ngrp - 1)

                # fresh keys (the only masked block)
                scn_ps = ps.tile([s, rows], F32, tag="sc", name=f"scn{bb}")
                nc.tensor.matmul(
                    scn_ps[:], kT_new[:, bb * s:(bb + 1) * s], qT_b32,
                    start=True, stop=True,
                )
                exn = exp_p.tile([s, rows], F32, tag="exn")
                nc.scalar.activation(exn[:], scn_ps[:], AF.Exp)
                nc.vector.tensor_mul(exn[:], exn[:], maskT)
                nc.vector.tensor_add(acc[0:s, :], acc[0:s, :], exn[:])
                nc.tensor.matmul(
                    attn_ps[:], v_new[bb][:], exn[:],
                    start=False, stop=True, skip_group_check=True,
                )
                # denominator: reduce acc over partitions, broadcast reciprocal
                dsum_ps = ps.tile([1, rows], F32, tag="sc", name=f"ds{bb}")
                nc.tensor.matmul(dsum_ps[:], ones_col[:], acc[:], start=True, stop=True)
                rden = den_p.tile([1, rows], F32, tag="rden")
                nc.vector.reciprocal(rden[:], dsum_ps[:])
                bc_ps = ps.tile([128, rows], F32, tag="sc", name=f"bc{bb}")
                nc.tensor.matmul(bc_ps[:], ones_row[:], rden[:], start=True, stop=True)
                rdenb = den_p.tile([128, rows], F32, tag="rdenb")
                nc.scalar.copy(rdenb[:], bc_ps[:])
                dst = attnT_sb[:].rearrange("p (j bb t) -> p j bb t", j=hq, bb=b)[
                    :, :, bb, :
                ]
                nc.vector.tensor_mul(
                    dst,
                    attn_ps[:].rearrange("p (j t) -> p j t", j=hq),
                    rdenb[:].rearrange("p (j t) -> p j t", j=hq),
                )

            # ---- o_proj: wo column-quarters stream on the sync queue (after kv),
            # out[tok, h] accumulated over j per 512-col block ----
            wo_dv = wo_d.rearrange("p (j m) -> p j m", j=hq)
            wo_t = []
            for q in range(h // 512):
                wt = w_p.tile([128, hq * 512], BF16, tag=f"wo{q}", name=f"wo{q}")
                nc.sync.dma_start(
                    wt[:].rearrange("p (j m) -> p j m", j=hq),
                    wo_dv[:, :, q * 512:(q + 1) * 512],
                )
                wo_t.append(wt)
            for nt in range(h // 512):
                o_ps = ps.tile([tok, 512], F32, tag="qt", bufs=4, name=f"o{nt}")
                wt = wo_t[nt][:].rearrange("p (j m) -> p j m", j=hq)
                for j in range(hq):
                    nc.tensor.matmul(
                        o_ps[:], attnT_sb[:, j * tok:(j + 1) * tok],
                        wt[:, j, 0:512],
                        start=(j == 0), stop=(j == hq - 1),
                    )
                o_sb = o_p.tile([tok, 512], BF16, tag="osb")
                nc.scalar.copy(o_sb[:], o_ps[:])
                # SP queue: keeps Act.SEQ free for the next block's copy
                nc.sync.dma_start(out_d[:, nt * 512:(nt + 1) * 512], o_sb[:])

    nc.compile()
    return nc


_NC_CACHE = {}


def _get_nc(key=(B, S, H, HQ, HD, PAST)):
    if key not in _NC_CACHE:
        _NC_CACHE[key] = build_nc(*key)
    return _NC_CACHE[key]


def make_in_maps(hidden_states, k_cache, v_cache, Wq, Wk, Wv, Wo, position_ids):
    """Host-side shard + layout prep: one input dict per core."""
    bf16 = ml_dtypes.bfloat16
    # hT[p, (c, t)] = hidden[t, c*128+p]
    hT = np.ascontiguousarray(
        hidden_states.reshape(TOK, H).T.astype(np.float32)
        .reshape(NCH, 128, TOK).transpose(1, 0, 2).reshape(128, NCH * TOK)
    ).astype(bf16)
    # RoPE tables in [d, (b, t)] layout, duplicated freq block (half-split rope)
    inv_freq = (1.0 / (ROPE_BASE ** (np.arange(0, HD, 2, dtype=np.float64) / HD)))
    ang = position_ids.astype(np.float64).reshape(-1)[None, :] * np.concatenate(
        [inv_freq, inv_freq]
    )[:, None]                                           # [hd, tok]
    blob = np.zeros((128, 4 * TOK), np.float32)
    blob[:, 0:TOK] = np.cos(ang)
    blob[:, TOK:2 * TOK] = np.sin(ang)
    blob[:, 2 * TOK:3 * TOK] = -np.sin(ang)
    # mask over fresh keys: mask[j, (h, t)] = 1 if j <= t (bottom-right causal)
    jj = np.arange(S)[:, None]
    tt = np.tile(np.arange(S)[None, :], (1, HQ)).reshape(1, ROWS)
    blob[0:S, 3 * TOK:3 * TOK + ROWS] = (jj <= tt).astype(np.float32)

    in_maps = []
    for c in range(NCORES):
        q0 = c * HQ * HD
        # wq[p, (ch, j*hd+d)] = Wq[ch*128+p, q0 + j*hd+d] * SCALE
        wq = np.ascontiguousarray(
            (Wq[:, q0:q0 + HQ * HD] * SCALE).astype(np.float32)
            .reshape(NCH, 128, HQ * HD).transpose(1, 0, 2).reshape(128, -1)
        ).astype(bf16)
        wkv = np.ascontiguousarray(
            np.concatenate(
                [Wk[:, c * HD:(c + 1) * HD], Wv[:, c * HD:(c + 1) * HD]], axis=1
            ).astype(np.float32)
            .reshape(NCH, 128, 2 * HD).transpose(1, 0, 2).reshape(128, -1)
        ).astype(bf16)
        # wo[p, (j, m)] = Wo[q0 + j*128+p, m]
        wo = np.ascontiguousarray(
            Wo[q0:q0 + HQ * HD, :].astype(np.float32)
            .reshape(HQ, 128, H).transpose(1, 0, 2).reshape(128, -1)
        ).astype(bf16)
        in_maps.append({
            "hT": hT,
            "wq": wq,
            "wkv": wkv,
            "wo": wo,
            "kT": np.ascontiguousarray(
                k_cache[:, :, c, :].transpose(0, 2, 1)).astype(bf16),
            # v_r[b, p, tt*HD+d] = v[b, tt*128+p, d] -> contiguous 8KB DMA rows
            "v": np.ascontiguousarray(
                v_cache[:, :, c, :].reshape(B, PAST // 128, 128, HD)
                .transpose(0, 2, 1, 3).reshape(B, 128, PAST)
            ).astype(bf16),
            "blob": blob,
        })
    return in_maps


def kernel(hidden_states, k_cache, v_cache, Wq, Wk, Wv, Wo, position_ids):
    nc = _get_nc()
    in_maps = make_in_maps(
        np.asarray(hidden_states), np.asarray(k_cache), np.asarray(v_cache),
        np.asarray(Wq), np.asarray(Wk), np.asarray(Wv), np.asarray(Wo),
        np.asarray(position_ids),
    )
    res = run_bass_kernel_spmd(nc, in_maps, list(range(NCORES)))
    out = np.zeros((TOK, H), np.float32)
    for c in range(NCORES):
        out += res.results[c]["out_p"].astype(np.float32)
    return out.reshape(B, S, H)


# revision 16
# speedup vs baseline: 1.4602x; 1.0029x over previous
"""Tensor-parallel Llama attention (decode, GQA, RoPE, KV-cache) on 8 TRN2 cores.

Sharding: core c owns kv-head c and q-heads 4c..4c+3. Wq/Wk/Wv are sharded
column-wise, Wo row-wise; each core computes a partial o_proj output and the
host sums the 8 partials (the all-reduce).

v2 layout notes (all driven by the serial DMA/HWDGE devices in the timeline
model: ~625ns fixed cost per DMA instruction + 360GB/s aggregate transfer):
  - Every DRAM tensor is host-pre-arranged to the exact SBUF tile layout so
    each load is one large contiguous-elem DMA (>=1KB runs). ~45 DMAs total.
  - All weights live in SBUF for the whole kernel (wq 32KB/part, wkv 16,
    wo 32); only the kv cache streams (1MB tiles, 3-deep per tag).
  - Weights+consts go on the Activation hwdge queue, hT+kv+wo on the SP
    queue, so kv streaming and weight loads interleave at the DMA device
    while staying ordered within each queue (wo after kv by queue order).
  - q projection is computed transposed (qT[j] = Wq_j.T @ hT chunks) so all
    proj matmuls use the full 128-partition output and no PE transposes or
    identity are needed; RoPE reads the qT PSUM tiles directly.
  - Scores stay [kpos, rows] with 8-tile PSUM groups (512-wide exp), the
    attn.V accumulation interleaves one group behind the scores, and the
    softmax denominator is a ones-column matmul + reciprocal broadcast.
  - PSUM budget (8 banks): qt(4): qT heads -> o_proj rotation; B(2):
    kT/v proj -> per-batch attn accumulators; sc(2): score groups, fresh
    scores, dsum, bc.
"""

import numpy as np
import ml_dtypes

import concourse.bass as bass
import concourse.mybir as mybir
import concourse.tile as tile
from concourse import bacc
from concourse.bass_utils import run_bass_kernel_spmd

F32 = mybir.dt.float32
BF16 = mybir.dt.bfloat16
AF = mybir.ActivationFunctionType

# Problem shape (hardcoded per contract)
B, S, H = 4, 16, 4096
NH, NKV, HD = 32, 8, 128
PAST = 8192
ROPE_BASE = 10000.0
NCORES = 8
HQ = NH // NCORES          # q heads per core = 4
TOK = B * S                # 64 tokens
NCH = H // 128             # 32 contraction chunks for projections
ROWS = HQ * S              # 64 (head, token) query rows per batch
SCALE = HD ** -0.5


def build_nc(b=B, s=S, h=H, hq=HQ, hd=HD, past=PAST):
    tok = b * s
    nch = h // 128
    rows = hq * s
    half_kv = past // 2                 # kv streamed in 2 x 1MB tiles per batch
    GRP = 8                             # kpos-tiles per score group (512 cols)
    KPG = GRP * 128                     # kpos per group = 1024
    ngrp = past // KPG                  # 8 groups per batch

    nc = bacc.Bacc("TRN2", target_bir_lowering=False, debug=False)

    # Host-side pre-arranged layouts (see make_in_maps)
    hT_d = nc.dram_tensor("hT", [128, nch * tok], BF16, kind="ExternalInput").ap()
    wq_d = nc.dram_tensor("wq", [128, nch * hq * hd], BF16, kind="ExternalInput").ap()
    wkv_d = nc.dram_tensor("wkv", [128, nch * 2 * hd], BF16, kind="ExternalInput").ap()
    wo_d = nc.dram_tensor("wo", [128, hq * h], BF16, kind="ExternalInput").ap()
    kT_d = nc.dram_tensor("kT", [b, 128, past], BF16, kind="ExternalInput").ap()
    v_d = nc.dram_tensor("v", [b, 128, past], BF16, kind="ExternalInput").ap()
    # const blob: cols 0:64 cosT, 64:128 sinT, 128:192 -sinT, 192:256 mask(rows 0:16)
    blob_d = nc.dram_tensor("blob", [128, 4 * tok], F32, kind="ExternalInput").ap()
    out_d = nc.dram_tensor("out_p", [tok, h], BF16, kind="ExternalOutput").ap()

    with tile.TileContext(nc) as tc:
        import contextlib

        with contextlib.ExitStack() as ctx:
            ep = ctx.enter_context
            const_p = ep(tc.tile_pool(name="const", bufs=1))
            w_p = ep(tc.tile_pool(name="w", bufs=1))
            kv_p = ep(tc.tile_pool(name="kv", bufs=3))
            qkv_p = ep(tc.tile_pool(name="qkv", bufs=1))
            rope_p = ep(tc.tile_pool(name="rope", bufs=2))
            exp_p = ep(tc.tile_pool(name="exp", bufs=6))
            acc_p = ep(tc.tile_pool(name="acc", bufs=2))
            den_p = ep(tc.tile_pool(name="den", bufs=2))
            o_p = ep(tc.tile_pool(name="o", bufs=8))
            ps = ep(tc.tile_pool(name="ps", bufs=2, space="PSUM"))

            # ---- consts (scalar/Act hwdge queue) ----
            blob = const_p.tile([128, 4 * tok], F32)
            nc.scalar.dma_start(blob[:], blob_d[:])
            cosT = blob[:, 0:tok]
            sinT = blob[:, tok:2 * tok]
            nsinT = blob[:, 2 * tok:3 * tok]
            maskT = blob[0:s, 3 * tok:3 * tok + rows]
            ones_col = const_p.tile([128, 1], F32)
            nc.vector.memset(ones_col[:], 1.0)
            ones_row = const_p.tile([1, 128], F32)
            nc.vector.memset(ones_row[:], 1.0)

            # ---- weights resident in SBUF; separate tiles per ~1MB DMA so
            # consumers only wait for their own quarter ----
            qw = nch * hq * hd // 4     # wq quarter cols (8 chunks each)
            hw_ = nch * 2 * hd // 2     # wkv half cols (16 chunks each)
            wq_t = [
                w_p.tile([128, qw], BF16, tag=f"wq{i}", name=f"wq{i}")
                for i in range(4)
            ]
            wkv_t = [
                w_p.tile([128, hw_], BF16, tag=f"wkv{i}", name=f"wkv{i}")
                for i in range(2)
            ]
            # All big loads share the SP queue in exact consumption order:
            # the scheduler keeps program order among ready DMAs, so weights
            # are guaranteed to precede the kv stream at the DMA device.
            hT = w_p.tile([128, nch * tok], BF16, tag="hT")
            nc.sync.dma_start(hT[:], hT_d[:])
            nc.sync.dma_start(wkv_t[0][:], wkv_d[:, 0:hw_])
            nc.sync.dma_start(wq_t[0][:], wq_d[:, 0:qw])
            nc.sync.dma_start(wq_t[1][:], wq_d[:, qw:2 * qw])
            nc.sync.dma_start(wkv_t[1][:], wkv_d[:, hw_:2 * hw_])
            nc.sync.dma_start(wq_t[2][:], wq_d[:, 2 * qw:3 * qw])
            nc.sync.dma_start(wq_t[3][:], wq_d[:, 3 * qw:4 * qw])

            # ---- projections, q transposed: qT_ps[j] = Wq_j.T @ h ----
            qT_ps = [
                ps.tile([hd, tok], F32, tag="qt", bufs=4, name=f"qt{j}")
                for j in range(hq)
            ]
            kT_ps = ps.tile([hd, tok], F32, tag="B")
            v_ps = ps.tile([tok, hd], F32, tag="B")
            for c in range(nch):
                rhs_h = hT[:, c * tok:(c + 1) * tok]
                fl = dict(start=(c == 0), stop=(c == nch - 1))
                wq_q = wq_t[c // 8]
                cq = (c % 8) * hq * hd
                wkv_h = wkv_t[c // 16]
                ch = (c % 16) * 2 * hd
                for j in range(hq):
                    nc.tensor.matmul(
                        qT_ps[j][:],
                        wq_q[:, cq + j * hd:cq + (j + 1) * hd],
                        rhs_h, **fl,
                    )
                nc.tensor.matmul(
                    kT_ps[:], wkv_h[:, ch:ch + hd], rhs_h, **fl
                )
                nc.tensor.matmul(
                    v_ps[:], rhs_h, wkv_h[:, ch + hd:ch + 2 * hd], **fl
                )

            # ---- RoPE -> qT_sb [128, (b,hq,s)], kT_new [128, (b,s)] ----
            half = hd // 2
            qT_sb = qkv_p.tile([128, b * rows], F32, tag="qT")
            kT_new = qkv_p.tile([128, tok], F32, tag="kTn")

            def rope(dst, src_ps):
                t1 = rope_p.tile([128, tok], F32, tag="r1")
                nc.vector.tensor_mul(t1[:], src_ps[:], cosT)
                t2 = rope_p.tile([128, tok], F32, tag="r2")
                nc.vector.tensor_mul(
                    t2[0:half, :], src_ps[half:hd, :], nsinT[0:half, :]
                )
                nc.vector.tensor_mul(
                    t2[half:hd, :], src_ps[0:half, :], sinT[half:hd, :]
                )
                nc.vector.tensor_add(dst, t1[:], t2[:])

            for j in range(hq):
                dst = qT_sb[:].rearrange("p (bb j t) -> p bb j t", bb=b, j=hq)[:, :, j, :]
                rope(dst, qT_ps[j][:])
            rope(kT_new[:], kT_ps[:])
            qT_bf = qkv_p.tile([128, b * rows], BF16, tag="qTbf")
            nc.vector.tensor_copy(qT_bf[:], qT_sb[:])

            # fresh v rows per batch at partition base 0 (PE stationary operand)
            v_sb = qkv_p.tile([tok, hd], F32, tag="vsb")
            nc.scalar.copy(v_sb[:], v_ps[:])
            v_new = [
                qkv_p.tile([s, hd], F32, tag=f"vnew{bb}", name=f"vnew{bb}")
                for bb in range(b)
            ]
            for bb in range(b):
                nc.scalar.dma_start(v_new[bb][:], v_sb[bb * s:(bb + 1) * s, :])

            # ---- attention per batch ----
            attnT_sb = qkv_p.tile([128, hq * tok], BF16, tag="attnT")  # (j, b, t)
            for bb in range(b):
                qT_b = qT_bf[:, bb * rows:(bb + 1) * rows]
                qT_b32 = qT_sb[:, bb * rows:(bb + 1) * rows]
                attn_ps = ps.tile([hd, rows], F32, tag="B", name=f"attn{bb}")
                acc = acc_p.tile([128, rows], F32, tag="acc")
                kts, vts = [], []
                for hf in range(2):
                    kt = kv_p.tile([128, half_kv], BF16, tag="kt", name=f"kt{bb}{hf}")
                    nc.sync.dma_start(
                        kt[:], kT_d[bb, :, hf * half_kv:(hf + 1) * half_kv]
                    )
                    kts.append(kt)
                    vt = kv_p.tile([128, half_kv], BF16, tag="vt", name=f"vt{bb}{hf}")
                    nc.sync.dma_start(
                        vt[:], v_d[bb, :, hf * half_kv:(hf + 1) * half_kv]
                    )
                    vts.append(vt)

                exs = [None] * ngrp

                def attn_group(g):
                    vt = vts[g // (ngrp // 2)]
                    off = (g % (ngrp // 2)) * KPG
                    for u in range(GRP):
                        nc.tensor.matmul(
                            attn_ps[:], vt[:, off + u * 128:off + (u + 1) * 128],
                            exs[g][:, u * rows:(u + 1) * rows],
                            start=(g == 0 and u == 0), stop=False,
                            skip_group_check=True,
                        )

                for g in range(ngrp):
                    kt = kts[g // (ngrp // 2)]
                    off = (g % (ngrp // 2)) * KPG
                    sc_ps = ps.tile([128, GRP * rows], F32, tag="sc", name=f"sc{bb}{g}")
                    for u in range(GRP):
                        nc.tensor.matmul(
                            sc_ps[:, u * rows:(u + 1) * rows],
                            kt[:, off + u * 128:off + (u + 1) * 128], qT_b,
                            start=(u == 0), stop=(u == GRP - 1),
                        )
                    ex = exp_p.tile([128, GRP * rows], BF16, tag="ex")
                    nc.scalar.activation(ex[:], sc_ps[:], AF.Exp)
                    exs[g] = ex
                    red = acc if g == 0 else acc_p.tile(
                        [128, rows], F32, tag="red", name="red")
                    nc.vector.tensor_reduce(
                        red[:],
                        ex[:].rearrange("p (u q) -> p q u", u=GRP),
                        axis=mybir.AxisListType.X, op=mybir.AluOpType.add,
                    )
                    if red is not acc:
                        nc.vector.tensor_add(acc[:], acc[:], red[:])
                    if g > 0:
                        attn_group(g - 1)
                attn_group(ngrp - 1)

                # fresh keys (the only masked block)
                scn_ps = ps.tile([s, rows], F32, tag="sc", name=f"scn{bb}")
                nc.tensor.matmul(
                    scn_ps[:], kT_new[:, bb * s:(bb + 1) * s], qT_b32,
                    start=True, stop=True,
                )
                exn = exp_p.tile([s, rows], F32, tag="exn")
                nc.scalar.activation(exn[:], scn_ps[:], AF.Exp)
                nc.vector.tensor_mul(exn[:], exn[:], maskT)
                nc.vector.tensor_add(acc[0:s, :], acc[0:s, :], exn[:])
                nc.tensor.matmul(
                    attn_ps[:], v_new[bb][:], exn[:],
                    start=False, stop=True, skip_group_check=True,
                )
                # denominator: reduce acc over partitions, broadcast reciprocal
                dsum_ps = ps.tile([1, rows], F32, tag="sc", name=f"ds{bb}")
                nc.tensor.matmul(dsum_ps[:], ones_col[:], acc[:], start=True, stop=True)
                rden = den_p.tile([1, rows], F32, tag="rden")
                nc.vector.reciprocal(rden[:], dsum_ps[:])
                bc_ps = ps.tile([128, rows], F32, tag="sc", name=f"bc{bb}")
                nc.tensor.matmul(bc_ps[:], ones_row[:], rden[:], start=True, stop=True)
                rdenb = den_p.tile([128, rows], F32, tag="rdenb")
                nc.scalar.copy(rdenb[:], bc_ps[:])
                dst = attnT_sb[:].rearrange("p (j bb t) -> p j bb t", j=hq, bb=b)[
                    :, :, bb, :
                ]
                nc.vector.tensor_mul(
                    dst,
                    attn_ps[:].rearrange("p (j t) -> p j t", j=hq),
                    rdenb[:].rearrange("p (j t) -> p j t", j=hq),
                )

            # ---- o_proj: wo column-quarters stream on the sync queue (after kv),
            # out[tok, h] accumulated over j per 512-col block ----
            wo_dv = wo_d.rearrange("p (j m) -> p j m", j=hq)
            # 512-col blocks; the final block split in half so the drain after
            # the last weight byte is a half-size matmul/copy/DMA chain
            segs = [(nt * 512, 512) for nt in range(h // 512 - 1)]
            segs += [(h - 512, 256), (h - 256, 256)]
            wo_t = []
            for i, (m0, mw) in enumerate(segs):
                wt = w_p.tile([128, hq * mw], BF16, tag=f"wo{i}", name=f"wo{i}")
                wv = wt[:].rearrange("p (j m) -> p j m", j=hq)
                nc.sync.dma_start(wv, wo_dv[:, :, m0:m0 + mw])
                wo_t.append(wv)
            for i, (m0, mw) in enumerate(segs):
                wv = wo_t[i]
                o_ps = ps.tile([tok, mw], F32, tag="qt", bufs=4, name=f"o{i}")
                for j in range(hq):
                    nc.tensor.matmul(
                        o_ps[:], attnT_sb[:, j * tok:(j + 1) * tok],
                        wv[:, j, 0:mw],
                        start=(j == 0), stop=(j == hq - 1),
                    )
                o_sb = o_p.tile([tok, mw], BF16, tag="osb")
                nc.scalar.copy(o_sb[:], o_ps[:])
                # SP queue: keeps Act.SEQ free for the next block's copy
                nc.sync.dma_start(out_d[:, m0:m0 + mw], o_sb[:])

    nc.compile()
    return nc


_NC_CACHE = {}


def _get_nc(key=(B, S, H, HQ, HD, PAST)):
    if key not in _NC_CACHE:
        _NC_CACHE[key] = build_nc(*key)
    return _NC_CACHE[key]


def make_in_maps(hidden_states, k_cache, v_cache, Wq, Wk, Wv, Wo, position_ids):
    """Host-side shard + layout prep: one input dict per core."""
    bf16 = ml_dtypes.bfloat16
    # hT[p, (c, t)] = hidden[t, c*128+p]
    hT = np.ascontiguousarray(
        hidden_states.reshape(TOK, H).T.astype(np.float32)
        .reshape(NCH, 128, TOK).transpose(1, 0, 2).reshape(128, NCH * TOK)
    ).astype(bf16)
    # RoPE tables in [d, (b, t)] layout, duplicated freq block (half-split rope)
    inv_freq = (1.0 / (ROPE_BASE ** (np.arange(0, HD, 2, dtype=np.float64) / HD)))
    ang = position_ids.astype(np.float64).reshape(-1)[None, :] * np.concatenate(
        [inv_freq, inv_freq]
    )[:, None]                                           # [hd, tok]
    blob = np.zeros((128, 4 * TOK), np.float32)
    blob[:, 0:TOK] = np.cos(ang)
    blob[:, TOK:2 * TOK] = np.sin(ang)
    blob[:, 2 * TOK:3 * TOK] = -np.sin(ang)
    # mask over fresh keys: mask[j, (h, t)] = 1 if j <= t (bottom-right causal)
    jj = np.arange(S)[:, None]
    tt = np.tile(np.arange(S)[None, :], (1, HQ)).reshape(1, ROWS)
    blob[0:S, 3 * TOK:3 * TOK + ROWS] = (jj <= tt).astype(np.float32)

    in_maps = []
    for c in range(NCORES):
        q0 = c * HQ * HD
        # wq[p, (ch, j*hd+d)] = Wq[ch*128+p, q0 + j*hd+d] * SCALE
        wq = np.ascontiguousarray(
            (Wq[:, q0:q0 + HQ * HD] * SCALE).astype(np.float32)
            .reshape(NCH, 128, HQ * HD).transpose(1, 0, 2).reshape(128, -1)
        ).astype(bf16)
        wkv = np.ascontiguousarray(
            np.concatenate(
                [Wk[:, c * HD:(c + 1) * HD], Wv[:, c * HD:(c + 1) * HD]], axis=1
            ).astype(np.float32)
            .reshape(NCH, 128, 2 * HD).transpose(1, 0, 2).reshape(128, -1)
        ).astype(bf16)
        # wo[p, (j, m)] = Wo[q0 + j*128+p, m]
        wo = np.ascontiguousarray(
            Wo[q0:q0 + HQ * HD, :].astype(np.float32)
            .reshape(HQ, 128, H).transpose(1, 0, 2).reshape(128, -1)
        ).astype(bf16)
        in_maps.append({
            "hT": hT,
            "wq": wq,
            "wkv": wkv,
            "wo": wo,
            "kT": np.ascontiguousarray(
                k_cache[:, :, c, :].transpose(0, 2, 1)).astype(bf16),
            # v_r[b, p, tt*HD+d] = v[b, tt*128+p, d] -> contiguous 8KB DMA rows
            "v": np.ascontiguousarray(
                v_cache[:, :, c, :].reshape(B, PAST // 128, 128, HD)
                .transpose(0, 2, 1, 3).reshape(B, 128, PAST)
            ).astype(bf16),
            "blob": blob,
        })
    return in_maps


def kernel(hidden_states, k_cache, v_cache, Wq, Wk, Wv, Wo, position_ids):
    nc = _get_nc()
    in_maps = make_in_maps(
        np.asarray(hidden_states), np.asarray(k_cache), np.asarray(v_cache),
        np.asarray(Wq), np.asarray(Wk), np.asarray(Wv), np.asarray(Wo),
        np.asarray(position_ids),
    )
    res = run_bass_kernel_spmd(nc, in_maps, list(range(NCORES)))
    out = np.zeros((TOK, H), np.float32)
    for c in range(NCORES):
        out += res.results[c]["out_p"].astype(np.float32)
    return out.reshape(B, S, H)
